# revision 1
# baseline (speedup 1.0000x reference)
"""Trainium2 Bass kernel for nn_MultiHeadAttention (B=2, S=2048, E=1024, H=16).

Sharding: 8 NeuronCores = data-parallel over the 2 batches x tensor-parallel
over the 16 heads in 4 groups of 4 heads (Wq/Wk/Wv split column-wise, Wo
row-wise).  Each core computes a full-[S, E] partial of its batch's output;
the host sums the 4 head-group partials per batch.

Per-core device algorithm (S.T orientation so exp(S.T) feeds P@V directly):
  Q.T/K.T[n, s] = (wT chunk).T @ xT chunk      (e-outer, chases input DMAs)
  V[s, n]       = (xvT chunk).T @ wvT chunk    stored as v_ext = [V_h | ones]
  S.T_h[k, q]   = (K_h.T chunk).T @ Q_h.T      row-packed head pairs (d=64)
  P.T           = exp(S.T / 8)                 one ACT op per (k, head pair)
  [O.T_h; sums] = (v_ext_h).T @ P.T_h          fused: PSUM rows 0-63 = O.T_h,
                                               rows 64-127 = rowsum broadcast
  O.Tn_h        = O.T_h * recip(sums)          recip shifted p64->p0 via DMA
  out[m, :]     = sum_h (oT_h chunk).T @ woT_h

dtypes: matmul inputs for the projections are fp16 (host pre-cast halves the
HBM traffic; 10-bit mantissa beats bf16 by 8x); everything SBUF-internal
(Q.T/K.T/V/P.T/O.T/Wo) is float32r (full fp32 bits, reduced-precision
multiply); accumulation is always fp32.
"""

import numpy as np
from contextlib import ExitStack

import ml_dtypes

import concourse.bass as bass
import concourse.mybir as mybir
import concourse.tile as tile
from concourse.tile import ScopedClock
from concourse.bass_utils import run_bass_kernel_spmd

# ---------------------------------------------------------------------------
# Workarounds for the walrus build on this stack, which rejects more than ONE
# semaphore wait per instruction ("Too many sync wait commands").
# ---------------------------------------------------------------------------
_orig_commit_instruction = tile.TileContext._commit_instruction


def _commit_instruction(self, inst, lazy_reg_writes=True):
    si = getattr(inst, "sync_info", None)
    if si is not None and si.on_wait and len(si.on_wait) > 1:
        waits = list(si.on_wait)
        for w in waits[:-1]:
            nop = mybir.InstNoOp(
                name=self.nc.get_next_instruction_name(),
                ins=[], outs=[], engine=inst.engine,
            )
            nop.bass_nofuse = True
            nop.sync_info = mybir.SyncInfo(on_wait=[w], on_update=[])
            _orig_commit_instruction(self, nop, lazy_reg_writes=False)
        inst.sync_info = mybir.SyncInfo(
            on_wait=[waits[-1]], on_update=list(si.on_update or [])
        )
    return _orig_commit_instruction(self, inst, lazy_reg_writes)


def _drain_and_barrier(self, tick_clock, wait_clock):
    nc = self.nc
    drain_inst = nc.sync.drain()
    wait_clock.add_sem_waits(
        drain_inst.ins, ScopedClock({None: tick_clock.global_clock})
    )
    si = drain_inst.ins.sync_info
    waits = list(si.on_wait) if si and si.on_wait else []
    if len(waits) > 1:
        drain_inst.ins.sync_info = mybir.SyncInfo(
            on_wait=waits[:1], on_update=list(si.on_update or [])
        )
        for w in waits[1:]:
            extra = nc.sync.drain()
            esi = extra.ins.sync_info
            extra.ins.sync_info = mybir.SyncInfo(
                on_wait=[w],
                on_update=list(esi.on_update or []) if esi else [],
            )
    nc.all_engine_barrier()
    assert self.sems is not None
    popped = nc._tile_sem_poison_stack.pop()
    assert popped is self._sem_poison
    nc.clear_and_free_semaphores(list(self.sems.allocated().values()))
    nc.all_engine_barrier()


def _apply_tilefix():
    tile.TileContext._commit_instruction = _commit_instruction
    tile.TileContext._drain_and_barrier = _drain_and_barrier


_apply_tilefix()

# ---------------------------------------------------------------------------
# Problem constants (hardcoded)
# ---------------------------------------------------------------------------
B, S, E, H = 2, 2048, 1024, 16
HC, D = 4, 64              # heads per core, head dim
NCORES = 8
NE = E // 128              # 8  e-chunks
NQ = S // 512              # 4  q-chunks
NK = S // 128              # 16 k-chunks
NM = S // 128              # 16 m-chunks

F32 = mybir.dt.float32
BF16 = mybir.dt.bfloat16
FP16 = mybir.dt.float16


def build(mmdt=mybir.dt.float32r, pdt=mybir.dt.float32r, xdt=FP16,
          ovbufs=3, xbufs=1, shift_eng="scalar", ptbufs=6):
    nc = bass.Bass()
    xqT = nc.dram_tensor("xqT", [E, S], xdt, kind="ExternalInput")
    xkT = nc.dram_tensor("xkT", [E, S], xdt, kind="ExternalInput")
    xvT = nc.dram_tensor("xvT", [E, S], xdt, kind="ExternalInput")
    wqT = nc.dram_tensor("wqT", [E, 256], xdt, kind="ExternalInput")
    wkT = nc.dram_tensor("wkT", [E, 256], xdt, kind="ExternalInput")
    wvT = nc.dram_tensor("wvT", [E, 256], xdt, kind="ExternalInput")
    woT = nc.dram_tensor("woT", [256, E], mmdt, kind="ExternalInput")
    vones = nc.dram_tensor("vones", [128, 256], mmdt, kind="ExternalInput")
    out = nc.dram_tensor("out", [S, E], F32, kind="ExternalOutput")

    with tile.TileContext(nc) as tc, ExitStack() as ctx:
        consts = ctx.enter_context(tc.tile_pool(name="consts", bufs=1))
        wpool = ctx.enter_context(tc.tile_pool(name="w", bufs=1))
        actpool = ctx.enter_context(tc.tile_pool(name="acts", bufs=1))
        xpool = ctx.enter_context(tc.tile_pool(name="x", bufs=10))

        # preload the exp table before the hot loop
        dummy = consts.tile([1, 8], F32)
        nc.vector.memset(dummy[:], 0.0)
        nc.scalar.activation(dummy[:], dummy[:], mybir.ActivationFunctionType.Exp)

        wv_sb = wpool.tile([128, NE, 256], xdt)
        wo_sb = wpool.tile([64, HC, E], mmdt)

        qT_sb = actpool.tile([128, 2, S], mmdt)        # [(2 heads x d), pair, s]
        kT_sb = actpool.tile([128, 2, S], mmdt)
        v_sb = actpool.tile([128, NK, HC, 128], mmdt)  # [s%128, k, h, V_h|ones]

        def proj_eouter(w_sb, xchunks, dst, psA):
            tiles = [psA.tile([128, 512], F32, tag="mm", name=f"pj{i}")
                     for i in range(8)]
            for e in range(NE):
                for nch in range(2):
                    for m in range(NQ):
                        nc.tensor.matmul(
                            tiles[nch * NQ + m][:],
                            w_sb[:, e, nch * 128:(nch + 1) * 128],
                            xchunks[e][:, m * 512:(m + 1) * 512],
                            start=(e == 0), stop=(e == NE - 1),
                        )
            for nch in range(2):
                for m in range(NQ):
                    nc.vector.tensor_copy(
                        dst[:, nch, m * 512:(m + 1) * 512],
                        tiles[nch * NQ + m][:])

        # ---- prefix: K then Q projections (e-outer, DMA-chasing) ----
        with tc.tile_pool(name="wprefix", bufs=1) as wprefix, \
             tc.tile_pool(name="psA", bufs=8, space="PSUM") as psA:
            wk_sb = wprefix.tile([128, NE, 256], xdt)
            wq_sb = wprefix.tile([128, NE, 256], xdt)
            nc.sync.dma_start(wk_sb[:], wkT.rearrange("(ec p) n -> p ec n", p=128))
            nc.sync.dma_start(wq_sb[:], wqT.rearrange("(ec p) n -> p ec n", p=128))

            xk = []
            for e in range(NE):
                t = xpool.tile([128, S], xdt, tag="xchunk", name=f"xk{e}")
                nc.sync.dma_start(t[:], xkT[e * 128:(e + 1) * 128, :])
                xk.append(t)
            xq = []
            for e in range(NE):
                t = xpool.tile([128, S], xdt, tag="xchunk", name=f"xq{e}")
                nc.sync.dma_start(t[:], xqT[e * 128:(e + 1) * 128, :])
                xq.append(t)

            proj_eouter(wk_sb, xk, kT_sb, psA)
            proj_eouter(wq_sb, xq, qT_sb, psA)

        # V-side loads stream in behind the prefix on the SP queue
        nc.sync.dma_start(wv_sb[:], wvT.rearrange("(ec p) n -> p ec n", p=128))
        nc.sync.dma_start(wo_sb[:], woT.rearrange("(h p) j -> p h j", p=64))
        for k in range(NK):
            nc.gpsimd.dma_start(
                v_sb[:, k, :, 64:128],
                vones.rearrange("p (h c) -> p h c", h=HC))
        xv = []
        for e in range(NE):
            t = xpool.tile([128, S], xdt, tag="xchunk", name=f"xv{e}")
            nc.sync.dma_start(t[:], xvT[e * 128:(e + 1) * 128, :])
            xv.append(t)

        # ---- steady state pools ----
        oTpool = ctx.enter_context(tc.tile_pool(name="oT", bufs=1))
        ppool = ctx.enter_context(tc.tile_pool(name="pT", bufs=ptbufs))
        rpool = ctx.enter_context(tc.tile_pool(name="recip", bufs=2))
        opool = ctx.enter_context(tc.tile_pool(name="outstage", bufs=2))
        psS = ctx.enter_context(tc.tile_pool(name="psS", bufs=2, space="PSUM"))
        psOV = ctx.enter_context(tc.tile_pool(name="psOV", bufs=ovbufs, space="PSUM"))
        psX = ctx.enter_context(tc.tile_pool(name="psX", bufs=xbufs, space="PSUM"))

        oT_sb = oTpool.tile([64, HC, S], mmdt)         # [d, h, s]

        def v_proj_tile(m):
            ps = psX.tile([128, 512], F32, tag="px", name=f"vp{m}")
            for e in range(NE):
                nc.tensor.matmul(
                    ps[:, 0:256],
                    xv[e][:, m * 128:(m + 1) * 128],
                    wv_sb[:, e, :],
                    start=(e == 0), stop=(e == NE - 1),
                )
            nc.vector.tensor_copy(
                v_sb[:, m, :, 0:64],
                ps[:, 0:256].rearrange("p (h c) -> p h c", h=HC))

        def out_proj_tile(m):
            stage = opool.tile([128, E], F32)
            for j in range(2):
                ps = psX.tile([128, 512], F32, tag="px", name=f"op{m}_{j}")
                for h in range(HC):
                    nc.tensor.matmul(
                        ps[:],
                        oT_sb[:, h, m * 128:(m + 1) * 128],
                        wo_sb[:, h, j * 512:(j + 1) * 512],
                        start=(h == 0), stop=(h == HC - 1),
                    )
                nc.vector.tensor_copy(stage[:, j * 512:(j + 1) * 512], ps[:])
            nc.gpsimd.dma_start(out[m * 128:(m + 1) * 128, :], stage[:])

        # V tiles are needed from the very first pass: emit them first
        for m in range(NM):
            v_proj_tile(m)

        for qc in range(NQ):
            qs = slice(qc * 512, (qc + 1) * 512)
            for pair in range(2):
                ps_ov = [psOV.tile([128, 512], F32, name=f"ov{i}", tag="ov")
                         for i in range(2)]
                for k in range(NK):
                    ks = slice(k * 128, (k + 1) * 128)
                    first, last = (k == 0), (k == NK - 1)
                    ps_s = psS.tile([128, 1024], F32)
                    # scores, row-packed: head A rows 0-63, head B rows 64-127
                    nc.tensor.matmul(ps_s[:, 0:512],
                                     kT_sb[0:64, pair, ks],
                                     qT_sb[0:64, pair, qs],
                                     start=True, stop=True)
                    nc.tensor.matmul(ps_s[:, 512:1024],
                                     kT_sb[64:128, pair, ks],
                                     qT_sb[64:128, pair, qs],
                                     start=True, stop=True)
                    # exp of both heads in one op; 1/sqrt(D) folded into scale
                    pT = ppool.tile([128, 1024], pdt)
                    nc.scalar.activation(pT[:], ps_s[:],
                                         mybir.ActivationFunctionType.Exp,
                                         scale=0.125)
                    # fused O.T + rowsum accumulation per head
                    for h2 in range(2):
                        h = pair * 2 + h2
                        nc.tensor.matmul(
                            ps_ov[h2][:],
                            v_sb[:, k, h, :],
                            pT[:, h2 * 512:(h2 + 1) * 512],
                            start=first, stop=last)
                # normalize: recip of sums (rows 64-127), shift to rows 0-63
                for h2 in range(2):
                    h = pair * 2 + h2
                    rt = rpool.tile([128, 512], F32, tag="rt")
                    nc.vector.reciprocal(rt[64:128, :], ps_ov[h2][64:128, :])
                    rb = rpool.tile([64, 512], F32, tag="rb")
                    getattr(nc, shift_eng).dma_start(rb[:], rt[64:128, :])
                    nc.vector.tensor_tensor(
                        oT_sb[:, h, qs], ps_ov[h2][0:64, :], rb[:],
                        mybir.AluOpType.mult)
            # out-proj for this q window (needs both pairs of this qc)
            for m in range(qc * 4, qc * 4 + 4):
                out_proj_tile(m)

    return nc


_NC_CACHE = {}


def _get_nc():
    if "nc" not in _NC_CACHE:
        _NC_CACHE["nc"] = build()
    return _NC_CACHE["nc"]


def _shard_inputs(query, key, value, Wq, Wk, Wv, Wo):
    """Host-side sharding + layout prep: core c = (batch c//4, head-group c%4)."""
    f16 = np.float16
    xT = []
    for b in range(B):
        xT.append((
            np.ascontiguousarray(query[b].T).astype(f16),
            np.ascontiguousarray(key[b].T).astype(f16),
            np.ascontiguousarray(value[b].T).astype(f16),
        ))
    wT = []
    for g in range(4):
        gc = slice(g * 256, (g + 1) * 256)
        wT.append((
            np.ascontiguousarray(Wq[gc].T).astype(f16),
            np.ascontiguousarray(Wk[gc].T).astype(f16),
            np.ascontiguousarray(Wv[gc].T).astype(f16),
            np.ascontiguousarray(Wo[:, gc].T),
        ))
    vones = np.ones((128, 256), dtype=np.float32)
    in_maps = []
    for c in range(NCORES):
        b, g = c // 4, c % 4
        qT, kT, vT = xT[b]
        wq, wk, wv, wo = wT[g]
        in_maps.append({
            "xqT": qT, "xkT": kT, "xvT": vT,
            "wqT": wq, "wkT": wk, "wvT": wv, "woT": wo,
            "vones": vones,
        })
    return in_maps


def kernel(query, key, value, Wq, Wk, Wv, Wo):
    query = np.asarray(query, dtype=np.float32)
    key = np.asarray(key, dtype=np.float32)
    value = np.asarray(value, dtype=np.float32)
    Wq = np.asarray(Wq, dtype=np.float32)
    Wk = np.asarray(Wk, dtype=np.float32)
    Wv = np.asarray(Wv, dtype=np.float32)
    Wo = np.asarray(Wo, dtype=np.float32)

    nc = _get_nc()
    in_maps = _shard_inputs(query, key, value, Wq, Wk, Wv, Wo)
    res = run_bass_kernel_spmd(nc, in_maps, core_ids=list(range(NCORES)))

    out = np.zeros((B, S, E), dtype=np.float32)
    for c in range(NCORES):
        out[c // 4] += res.results[c]["out"]
    return out



# revision 10
# speedup vs baseline: 1.3692x; 1.3692x over previous
"""Trainium2 Bass kernel for nn_MultiHeadAttention (B=2, S=2048, E=1024, H=16).

Sharding: 8 NeuronCores = data-parallel over the 2 batches x tensor-parallel
over the 16 heads in 4 groups of 4 heads (Wq/Wk/Wv split column-wise, Wo
row-wise).  Each core computes a full-[S, E] partial of its batch's output;
the host sums the 4 head-group partials per batch.

Per-core device algorithm (v2 — stationary-P orientation):
  K.T/Q.T[n, s]  = (wT chunk).T @ xT chunk       e-outer, chases input DMAs
  V[s, n]        = (xvT chunk).T @ wvT chunk     stored [s%128, k, h, V|1|pad]
  S.T_h[k, q]    = (K_h.T chunk).T @ Q_h.T       row-packed head pairs (d=64)
  P.T            = exp(S.T / 8)                  fp16, one ACT op per (k, pair)
  [O_h | rowsum] = (P.T chunk as STATIONARY).T @ [V_h | 1]
                   — moving dim is only 65 wide, so P@V costs half the
                   PE rows of the d-moving orientation
  O_hn           = O_h * recip(rowsum)           per-partition scalar on DVE
  O.T            = PE-transpose(O_hn)            fp16, 128 rows per 64x128 tile
  out[m, :]      = sum_pair (O.T pair chunk).T @ Wo.T pair chunk
                   — 2-head-packed contraction (K=128)

Everything fp16 except PSUM accumulation (fp32) and the reciprocal.  The
output partial is returned fp16 and summed on the host in fp32.
"""

import numpy as np
from contextlib import ExitStack

import concourse.bass as bass
import concourse.mybir as mybir
import concourse.tile as tile
from concourse.tile import ScopedClock
from concourse.bass_utils import run_bass_kernel_spmd
from concourse.masks import make_identity

# ---------------------------------------------------------------------------
# Workarounds for the walrus build on this stack, which rejects more than ONE
# semaphore wait per instruction ("Too many sync wait commands").
# ---------------------------------------------------------------------------
_orig_commit_instruction = tile.TileContext._commit_instruction


def _commit_instruction(self, inst, lazy_reg_writes=True):
    si = getattr(inst, "sync_info", None)
    if si is not None and si.on_wait and len(si.on_wait) > 1:
        waits = list(si.on_wait)
        for w in waits[:-1]:
            nop = mybir.InstNoOp(
                name=self.nc.get_next_instruction_name(),
                ins=[], outs=[], engine=inst.engine,
            )
            nop.bass_nofuse = True
            nop.sync_info = mybir.SyncInfo(on_wait=[w], on_update=[])
            _orig_commit_instruction(self, nop, lazy_reg_writes=False)
        inst.sync_info = mybir.SyncInfo(
            on_wait=[waits[-1]], on_update=list(si.on_update or [])
        )
    return _orig_commit_instruction(self, inst, lazy_reg_writes)


def _drain_and_barrier(self, tick_clock, wait_clock):
    nc = self.nc
    drain_inst = nc.sync.drain()
    wait_clock.add_sem_waits(
        drain_inst.ins, ScopedClock({None: tick_clock.global_clock})
    )
    si = drain_inst.ins.sync_info
    waits = list(si.on_wait) if si and si.on_wait else []
    if len(waits) > 1:
        drain_inst.ins.sync_info = mybir.SyncInfo(
            on_wait=waits[:1], on_update=list(si.on_update or [])
        )
        for w in waits[1:]:
            extra = nc.sync.drain()
            esi = extra.ins.sync_info
            extra.ins.sync_info = mybir.SyncInfo(
                on_wait=[w],
                on_update=list(esi.on_update or []) if esi else [],
            )
    nc.all_engine_barrier()
    assert self.sems is not None
    popped = nc._tile_sem_poison_stack.pop()
    assert popped is self._sem_poison
    nc.clear_and_free_semaphores(list(self.sems.allocated().values()))
    nc.all_engine_barrier()


def _apply_tilefix():
    tile.TileContext._commit_instruction = _commit_instruction
    tile.TileContext._drain_and_barrier = _drain_and_barrier


_apply_tilefix()

# ---------------------------------------------------------------------------
# Problem constants (hardcoded)
# ---------------------------------------------------------------------------
B, S, E, H = 2, 2048, 1024, 16
HC, D = 4, 64              # heads per core, head dim
NCORES = 8
NE = E // 128              # 8  e-chunks
NQC = S // 512             # 4  qc windows
NK = S // 128              # 16 k-chunks (also m-chunks)
NU = 2 * NQC               # 8 units = (qc, pair)

F32 = mybir.dt.float32
FP16 = mybir.dt.float16


def build(ptbufs=34, debug=False):
    nc = bass.Bass()
    xqT = nc.dram_tensor("xqT", [E, S], FP16, kind="ExternalInput")
    xkT = nc.dram_tensor("xkT", [E, S], FP16, kind="ExternalInput")
    xvT = nc.dram_tensor("xvT", [E, S], FP16, kind="ExternalInput")
    wqT = nc.dram_tensor("wqT", [E, 256], FP16, kind="ExternalInput")
    wkT = nc.dram_tensor("wkT", [E, 256], FP16, kind="ExternalInput")
    wvT = nc.dram_tensor("wvT", [E, 256], FP16, kind="ExternalInput")
    woT = nc.dram_tensor("woT", [256, E], FP16, kind="ExternalInput")
    out = nc.dram_tensor("out", [S, E], FP16, kind="ExternalOutput")
    if debug:
        dbg_qT = nc.dram_tensor("dbg_qT", [128, 2, S], FP16, kind="ExternalOutput")
        dbg_kT = nc.dram_tensor("dbg_kT", [128, 2, S], FP16, kind="ExternalOutput")
        dbg_v = nc.dram_tensor("dbg_v", [128, NK, HC, 72], FP16, kind="ExternalOutput")
        dbg_oT = nc.dram_tensor("dbg_oT", [128, 2, S], FP16, kind="ExternalOutput")
        dbg_pt = nc.dram_tensor("dbg_pt", [128, 1024], FP16, kind="ExternalOutput")

    with tile.TileContext(nc) as tc, ExitStack() as ctx:
        consts = ctx.enter_context(tc.tile_pool(name="consts", bufs=1))
        wpool = ctx.enter_context(tc.tile_pool(name="w", bufs=1))
        actpool = ctx.enter_context(tc.tile_pool(name="acts", bufs=1))
        xqpool = ctx.enter_context(tc.tile_pool(name="xq", bufs=8))
        xvpool = ctx.enter_context(tc.tile_pool(name="xv", bufs=8))

        # exp table preload + transpose identity
        dummy = consts.tile([1, 8], F32)
        nc.vector.memset(dummy[:], 0.0)
        nc.scalar.activation(dummy[:], dummy[:], mybir.ActivationFunctionType.Exp)
        ident = consts.tile([128, 128], FP16)
        make_identity(nc, ident)

        wq_sb = wpool.tile([128, NE, 256], FP16)
        wv_sb = wpool.tile([128, NE, 256], FP16)
        wo_sb = wpool.tile([128, 2, E], FP16)       # [n%128, pair, e]

        qT_sb = actpool.tile([128, 2, S], FP16)     # [(2 heads x d), pair, q]
        kT_sb = actpool.tile([128, 2, S], FP16)
        v_sb = actpool.tile([128, NK, HC, 72], FP16)  # [s%128, k, h, V|1|pad]
        oT_sb = actpool.tile([128, 2, S], FP16)     # [n%128, pair, s]

        # ---- DMA stream (sync queue), priority order ----
        # wk | xk | wq | xq[qc0 cols] | wv | xv | xq[rest cols] | wo
        with tc.tile_pool(name="xk", bufs=8) as xkpool, \
             tc.tile_pool(name="wk", bufs=1) as wkpool, \
             tc.tile_pool(name="psA", bufs=8, space="PSUM") as psA:
            wk_sb = wkpool.tile([128, NE, 256], FP16)
            nc.sync.dma_start(wk_sb[:], wkT.rearrange("(ec p) n -> p ec n", p=128))
            xk = []
            for e in range(NE):
                t = xkpool.tile([128, S], FP16, tag="xk", name=f"xk{e}")
                nc.sync.dma_start(t[:], xkT[e * 128:(e + 1) * 128, :])
                xk.append(t)
            nc.sync.dma_start(wq_sb[:], wqT.rearrange("(ec p) n -> p ec n", p=128))
            xq = []
            for e in range(NE):
                t = xqpool.tile([128, S], FP16, tag="xq", name=f"xq{e}")
                nc.sync.dma_start(t[:, 0:512], xqT[e * 128:(e + 1) * 128, 0:512])
                xq.append(t)
            nc.sync.dma_start(wv_sb[:], wvT.rearrange("(ec p) n -> p ec n", p=128))
            xv = []
            for e in range(NE):
                t = xvpool.tile([128, S], FP16, tag="xv", name=f"xv{e}")
                nc.sync.dma_start(t[:], xvT[e * 128:(e + 1) * 128, :])
                xv.append(t)
            for e in range(NE):
                nc.sync.dma_start(xq[e][:, 512:S], xqT[e * 128:(e + 1) * 128, 512:S])
            nc.sync.dma_start(wo_sb[:], woT.rearrange("(pr p) j -> p pr j", p=128))

            # ones column of v_ext (before any pv)
            nc.vector.memset(v_sb[:, :, :, 64:65], 1.0)

            # ---- prefix: K proj (all), Q proj (qc0) — e-outer, DMA-chasing
            ktiles = [psA.tile([128, 512], F32, tag="mm", name=f"kp{i}")
                      for i in range(8)]
            for e in range(NE):
                for nch in range(2):
                    for m in range(4):
                        nc.tensor.matmul(
                            ktiles[nch * 4 + m][:],
                            wk_sb[:, e, nch * 128:(nch + 1) * 128],
                            xk[e][:, m * 512:(m + 1) * 512],
                            start=(e == 0), stop=(e == NE - 1),
                        )
            for nch in range(2):
                for m in range(4):
                    nc.vector.tensor_copy(
                        kT_sb[:, nch, m * 512:(m + 1) * 512],
                        ktiles[nch * 4 + m][:])
            qtiles = [psA.tile([128, 512], F32, tag="mm", name=f"qp{i}")
                      for i in range(2)]
            for e in range(NE):
                for nch in range(2):
                    nc.tensor.matmul(
                        qtiles[nch][:],
                        wq_sb[:, e, nch * 128:(nch + 1) * 128],
                        xq[e][:, 0:512],
                        start=(e == 0), stop=(e == NE - 1),
                    )
            for nch in range(2):
                nc.vector.tensor_copy(qT_sb[:, nch, 0:512], qtiles[nch][:])

        # ---- steady-state pools (8 PSUM banks total) ----
        ptpool = ctx.enter_context(tc.tile_pool(name="pt", bufs=ptbufs))
        rpool = ctx.enter_context(tc.tile_pool(name="recip", bufs=4))
        onpool = ctx.enter_context(tc.tile_pool(name="onorm", bufs=4))
        stpool = ctx.enter_context(tc.tile_pool(name="stage", bufs=3))
        psS = ctx.enter_context(tc.tile_pool(name="psS", bufs=2, space="PSUM"))
        psO = ctx.enter_context(tc.tile_pool(name="psO", bufs=2, space="PSUM"))
        psX = ctx.enter_context(tc.tile_pool(name="psX", bufs=2, space="PSUM"))

        pt_tiles = {}     # (u, k) -> pT tile
        psO_tiles = {}    # u -> [h2=0 tile, h2=1 tile]
        on_tiles = {}     # (u, h2) -> normalized O tile

        def sc_exp(u, k):
            qc, pair = u >> 1, u & 1
            qs = slice(qc * 512, (qc + 1) * 512)
            ks = slice(k * 128, (k + 1) * 128)
            ps = psS.tile([128, 1024], F32, tag="ss", name=f"ss{u}_{k}")
            nc.tensor.matmul(ps[:, 0:512], kT_sb[0:64, pair, ks],
                             qT_sb[0:64, pair, qs], start=True, stop=True)
            nc.tensor.matmul(ps[:, 512:1024], kT_sb[64:128, pair, ks],
                             qT_sb[64:128, pair, qs], start=True, stop=True)
            pt = ptpool.tile([128, 1024], FP16, tag="pt", name=f"pt{u}_{k}")
            nc.scalar.activation(pt[:], ps[:],
                                 mybir.ActivationFunctionType.Exp, scale=0.125)
            pt_tiles[(u, k)] = pt

        def norm(u):
            """DVE: recip rowsums, scale O, write fp16 O_norm to SBUF."""
            for h2 in range(2):
                po = psO_tiles[u][h2]
                rt = rpool.tile([128, 4], F32, tag="rt", name=f"rt{u}_{h2}")
                nc.vector.reciprocal(rt[:], po[:, :, 64:65])
                on = onpool.tile([128, 4, 64], FP16, tag="on",
                                 name=f"on{u}_{h2}")
                for q in range(4):
                    nc.vector.tensor_scalar_mul(
                        on[:, q, :], po[:, q, 0:64], rt[:, q:q + 1])
                on_tiles[(u, h2)] = on
            del psO_tiles[u]

        def pv_group(u, g):
            """One accumulation group g = (h2, q): 16 k-chunk matmuls into
            psO[h2][:, q].  Only one group is ever open per PSUM bank."""
            qc, pair = u >> 1, u & 1
            h2, q = g >> 2, g & 3
            if g == 0:
                psO_tiles[u] = [
                    psO.tile([128, 4, 128], F32, tag="oo", name=f"oo{u}_{hh}")
                    for hh in range(2)]
            h = pair * 2 + h2
            for k in range(NK):
                nc.tensor.matmul(
                    psO_tiles[u][h2][:, q, 0:65],
                    pt_tiles[(u, k)][:, h2 * 512 + q * 128:h2 * 512 + (q + 1) * 128],
                    v_sb[:, k, h, 0:65],
                    start=(k == 0), stop=(k == NK - 1),
                )
            if g == 7:
                for k in range(NK):
                    pt_tiles.pop((u, k))
                norm(u)

        def transp(u):
            qc, pair = u >> 1, u & 1
            pst = psX.tile([128, 4, 128], FP16, tag="px", name=f"tp{u}")
            for h2 in range(2):
                on = on_tiles.pop((u, h2))
                for q in range(4):
                    nc.tensor.transpose(
                        pst[h2 * 64:(h2 + 1) * 64, q, :], on[:, q, :], ident[:])
            nc.vector.tensor_copy(
                oT_sb[:, pair, qc * 512:(qc + 1) * 512], pst[:, :, :])

        def vproj(m):
            ps = psX.tile([128, 512], F32, tag="px", name=f"vp{m}")
            for e in range(NE):
                nc.tensor.matmul(
                    ps[:, 0:256],
                    xv[e][:, m * 128:(m + 1) * 128],
                    wv_sb[:, e, :],
                    start=(e == 0), stop=(e == NE - 1),
                )
            nc.vector.tensor_copy(
                v_sb[:, m, :, 0:64],
                ps[:, 0:256].rearrange("p (h c) -> p h c", h=HC))

        def qproj(qc):
            qs = slice(qc * 512, (qc + 1) * 512)
            tiles = [psX.tile([128, 512], F32, tag="px", name=f"qp{qc}_{n}")
                     for n in range(2)]
            for e in range(NE):
                for nch in range(2):
                    nc.tensor.matmul(
                        tiles[nch][:],
                        wq_sb[:, e, nch * 128:(nch + 1) * 128],
                        xq[e][:, qs],
                        start=(e == 0), stop=(e == NE - 1),
                    )
            for nch in range(2):
                nc.vector.tensor_copy(qT_sb[:, nch, qs], tiles[nch][:])

        stage_tiles = {}

        def outproj(qc, m, j):
            """One (m, j) item: psX accum over pairs, stage copy, DMA."""
            mm = qc * 4 + m
            ps = psX.tile([128, 512], F32, tag="px", name=f"op{mm}_{j}")
            for pair in range(2):
                nc.tensor.matmul(
                    ps[:],
                    oT_sb[:, pair, mm * 128:(mm + 1) * 128],
                    wo_sb[:, pair, j * 512:(j + 1) * 512],
                    start=(pair == 0), stop=(pair == 1),
                )
            if j == 0:
                stage_tiles[mm] = stpool.tile([128, E], FP16, tag="st",
                                              name=f"st{mm}")
            nc.vector.tensor_copy(
                stage_tiles[mm][:, j * 512:(j + 1) * 512], ps[:])
            if j == 1:
                nc.sync.dma_start(out[mm * 128:(mm + 1) * 128, :],
                                  stage_tiles.pop(mm)[:])

        # ---- window schedule ----
        # slot -> list of extra emitters, per window (unit u scored in w=u)
        def op_items(qc):
            return [(lambda q=qc, m=m, j=j: outproj(q, m, j))
                    for m in range(4) for j in range(2)]

        extras = {u: {} for u in range(NU)}

        def put(u, slot, fn):
            extras[u].setdefault(slot, []).append(fn)

        # vproj must fully finish before the first pv group (each group reads
        # all 16 v chunks): m0-4 late in w0 (xv lands ~2/3 through w0),
        # m5-15 across w1.
        for i, m in enumerate(range(0, 5)):
            put(0, 11 + i, lambda m=m: vproj(m))
        for i, m in enumerate(range(5, 16)):
            put(1, 1 + i, lambda m=m: vproj(m))
        put(1, 13, lambda: qproj(1))
        put(3, 2, lambda: transp(0))
        put(3, 10, lambda: transp(1))
        put(3, 13, lambda: qproj(2))
        put(4, 1, lambda: transp(2))
        for i, fn in enumerate(op_items(0)):     # w4: outproj qc0
            put(4, 3 + (i & 6) + (i & 1), fn)    # slots 3,4,5,6,7,8,9,10 -> odd-ish spread
        put(4, 14, lambda: qproj(3))
        put(5, 1, lambda: transp(3))
        for i, fn in enumerate(op_items(1)):
            put(5, 3 + i, fn)
        put(6, 1, lambda: transp(4))
        put(7, 1, lambda: transp(5))
        for i, fn in enumerate(op_items(2)):
            put(7, 3 + i, fn)

        # pv groups per window: w2: u0 at even slots; w3: u1 slots 0-7,
        # u2 slots 8-15 (catch-up); w4+: u_{w-1} at even slots.
        def pv_items(u, k):
            if u == 2:
                return [(0, k >> 1)] if (k & 1) == 0 else []
            if u == 3:
                return [(1, k)] if k < 8 else [(2, k - 8)]
            if u >= 4:
                return [(u - 1, k >> 1)] if (k & 1) == 0 else []
            return []

        for u in range(NU):
            for k in range(NK):
                sc_exp(u, k)
                for (pu, g) in pv_items(u, k):
                    pv_group(pu, g)
                for fn in extras[u].get(k, ()):
                    fn()

        # ---- tail: pv(u7) after the last exps, then transposes + qc3 out
        if debug:
            nc.sync.dma_start(dbg_pt[:, :], pt_tiles[(7, NK - 1)][:])
        for g in range(8):
            pv_group(7, g)
        transp(6)
        transp(7)
        for fn in op_items(3):
            fn()

        if debug:
            nc.sync.dma_start(dbg_qT[:, :, :], qT_sb[:])
            nc.sync.dma_start(dbg_kT[:, :, :], kT_sb[:])
            nc.sync.dma_start(dbg_v[:, :, :, :], v_sb[:])
            nc.sync.dma_start(dbg_oT[:, :, :], oT_sb[:])

    return nc


_NC_CACHE = {}


def _get_nc():
    if "nc" not in _NC_CACHE:
        _NC_CACHE["nc"] = build()
    return _NC_CACHE["nc"]


def _shard_inputs(query, key, value, Wq, Wk, Wv, Wo):
    """Host-side sharding + layout prep: core c = (batch c//4, head-group c%4)."""
    f16 = np.float16
    xT = []
    for b in range(B):
        xT.append((
            np.ascontiguousarray(query[b].T).astype(f16),
            np.ascontiguousarray(key[b].T).astype(f16),
            np.ascontiguousarray(value[b].T).astype(f16),
        ))
    wT = []
    for g in range(4):
        gc = slice(g * 256, (g + 1) * 256)
        wT.append((
            np.ascontiguousarray(Wq[gc].T).astype(f16),
            np.ascontiguousarray(Wk[gc].T).astype(f16),
            np.ascontiguousarray(Wv[gc].T).astype(f16),
            np.ascontiguousarray(Wo[:, gc].T).astype(f16),
        ))
    in_maps = []
    for c in range(NCORES):
        b, g = c // 4, c % 4
        qT, kT, vT = xT[b]
        wq, wk, wv, wo = wT[g]
        in_maps.append({
            "xqT": qT, "xkT": kT, "xvT": vT,
            "wqT": wq, "wkT": wk, "wvT": wv, "woT": wo,
        })
    return in_maps


def kernel(query, key, value, Wq, Wk, Wv, Wo):
    query = np.asarray(query, dtype=np.float32)
    key = np.asarray(key, dtype=np.float32)
    value = np.asarray(value, dtype=np.float32)
    Wq = np.asarray(Wq, dtype=np.float32)
    Wk = np.asarray(Wk, dtype=np.float32)
    Wv = np.asarray(Wv, dtype=np.float32)
    Wo = np.asarray(Wo, dtype=np.float32)

    nc = _get_nc()
    in_maps = _shard_inputs(query, key, value, Wq, Wk, Wv, Wo)
    res = run_bass_kernel_spmd(nc, in_maps, core_ids=list(range(NCORES)))

    out = np.zeros((B, S, E), dtype=np.float32)
    for c in range(NCORES):
        out[c // 4] += res.results[c]["out"].astype(np.float32)
    return out


# revision 34
# speedup vs baseline: 1.3726x; 1.0025x over previous
"""Trainium2 Bass kernel for nn_MultiHeadAttention (B=2, S=2048, E=1024, H=16).

Sharding: 8 NeuronCores = data-parallel over the 2 batches x tensor-parallel
over the 16 heads in 4 groups of 4 heads (Wq/Wk/Wv split column-wise, Wo
row-wise).  Each core computes a full-[S, E] partial of its batch's output;
the host sums the 4 head-group partials per batch.

Per-core device algorithm (v2 — stationary-P orientation):
  K.T/Q.T[n, s]  = (wT chunk).T @ xT chunk       e-outer, chases input DMAs
  V[s, n]        = (xvT chunk).T @ wvT chunk     stored [s%128, k, h, V|1|pad]
  S.T_h[k, q]    = (K_h.T chunk).T @ Q_h.T       row-packed head pairs (d=64)
  P.T            = exp(S.T / 8)                  fp16, one ACT op per (k, pair)
  [O_h | rowsum] = (P.T chunk as STATIONARY).T @ [V_h | 1]
                   — moving dim is only 65 wide, so P@V costs half the
                   PE rows of the d-moving orientation
  O_hn           = O_h * recip(rowsum)           per-partition scalar on DVE
  O.T            = PE-transpose(O_hn)            fp16, 128 rows per 64x128 tile
  out[m, :]      = sum_pair (O.T pair chunk).T @ Wo.T pair chunk
                   — 2-head-packed contraction (K=128)

Everything fp16 except PSUM accumulation (fp32) and the reciprocal.  The
output partial is returned fp16 and summed on the host in fp32.
"""

import numpy as np
from contextlib import ExitStack

import concourse.bass as bass
import concourse.mybir as mybir
import concourse.tile as tile
from concourse.tile import ScopedClock
from concourse.bass import broadcast_tensor_aps
from concourse.bass_utils import run_bass_kernel_spmd
from concourse.masks import make_identity

# ---------------------------------------------------------------------------
# Workarounds for the walrus build on this stack, which rejects more than ONE
# semaphore wait per instruction ("Too many sync wait commands").
# ---------------------------------------------------------------------------
_orig_commit_instruction = tile.TileContext._commit_instruction


def _commit_instruction(self, inst, lazy_reg_writes=True):
    si = getattr(inst, "sync_info", None)
    if si is not None and si.on_wait and len(si.on_wait) > 1:
        waits = list(si.on_wait)
        for w in waits[:-1]:
            nop = mybir.InstNoOp(
                name=self.nc.get_next_instruction_name(),
                ins=[], outs=[], engine=inst.engine,
            )
            nop.bass_nofuse = True
            nop.sync_info = mybir.SyncInfo(on_wait=[w], on_update=[])
            _orig_commit_instruction(self, nop, lazy_reg_writes=False)
        inst.sync_info = mybir.SyncInfo(
            on_wait=[waits[-1]], on_update=list(si.on_update or [])
        )
    return _orig_commit_instruction(self, inst, lazy_reg_writes)


def _drain_and_barrier(self, tick_clock, wait_clock):
    nc = self.nc
    drain_inst = nc.sync.drain()
    wait_clock.add_sem_waits(
        drain_inst.ins, ScopedClock({None: tick_clock.global_clock})
    )
    si = drain_inst.ins.sync_info
    waits = list(si.on_wait) if si and si.on_wait else []
    if len(waits) > 1:
        drain_inst.ins.sync_info = mybir.SyncInfo(
            on_wait=waits[:1], on_update=list(si.on_update or [])
        )
        for w in waits[1:]:
            extra = nc.sync.drain()
            esi = extra.ins.sync_info
            extra.ins.sync_info = mybir.SyncInfo(
                on_wait=[w],
                on_update=list(esi.on_update or []) if esi else [],
            )
    nc.all_engine_barrier()
    assert self.sems is not None
    popped = nc._tile_sem_poison_stack.pop()
    assert popped is self._sem_poison
    nc.clear_and_free_semaphores(list(self.sems.allocated().values()))
    nc.all_engine_barrier()


def _apply_tilefix():
    tile.TileContext._commit_instruction = _commit_instruction
    tile.TileContext._drain_and_barrier = _drain_and_barrier


_apply_tilefix()

# ---------------------------------------------------------------------------
# Problem constants (hardcoded)
# ---------------------------------------------------------------------------
B, S, E, H = 2, 2048, 1024, 16
HC, D = 4, 64              # heads per core, head dim
NCORES = 8
NE = E // 128              # 8  e-chunks
NQC = S // 512             # 4  qc windows
NK = S // 128              # 16 k-chunks (also m-chunks)
NU = 2 * NQC               # 8 units = (qc, pair)

F32 = mybir.dt.float32
FP16 = mybir.dt.float16


def build(ptbufs=35, n_warm=37, debug=False):
    nc = bass.Bass()
    xqT = nc.dram_tensor("xqT", [E, S], FP16, kind="ExternalInput")
    xkT = nc.dram_tensor("xkT", [E, S], FP16, kind="ExternalInput")
    xvT = nc.dram_tensor("xvT", [E, S], FP16, kind="ExternalInput")
    wqT = nc.dram_tensor("wqT", [E, 256], FP16, kind="ExternalInput")
    wkT = nc.dram_tensor("wkT", [E, 256], FP16, kind="ExternalInput")
    wvT = nc.dram_tensor("wvT", [E, 256], FP16, kind="ExternalInput")
    woT = nc.dram_tensor("woT", [256, E], FP16, kind="ExternalInput")
    out = nc.dram_tensor("out", [S, E], FP16, kind="ExternalOutput")
    if debug:
        dbg_qT = nc.dram_tensor("dbg_qT", [128, 2, S], FP16, kind="ExternalOutput")
        dbg_kT = nc.dram_tensor("dbg_kT", [128, 2, S], FP16, kind="ExternalOutput")
        dbg_v = nc.dram_tensor("dbg_v", [128, NK, HC, 72], FP16, kind="ExternalOutput")
        dbg_oT = nc.dram_tensor("dbg_oT", [128, 2, S], FP16, kind="ExternalOutput")
        dbg_pt = nc.dram_tensor("dbg_pt", [128, 1024], FP16, kind="ExternalOutput")

    with tile.TileContext(nc) as tc, ExitStack() as ctx:
        consts = ctx.enter_context(tc.tile_pool(name="consts", bufs=1))
        wpool = ctx.enter_context(tc.tile_pool(name="w", bufs=1))
        actpool = ctx.enter_context(tc.tile_pool(name="acts", bufs=1))
        xqpool = ctx.enter_context(tc.tile_pool(name="xq", bufs=2))
        xvpool = ctx.enter_context(tc.tile_pool(name="xv", bufs=8))

        # exp table preload + transpose identity
        dummy = consts.tile([1, 8], F32)
        nc.vector.memset(dummy[:], 0.0)
        nc.scalar.activation(dummy[:], dummy[:], mybir.ActivationFunctionType.Exp)
        ident = consts.tile([128, 128], FP16)
        make_identity(nc, ident)
        warm_in = consts.tile([128, 512], FP16)
        nc.gpsimd.memset(warm_in[:], 0.0)

        wq_sb = wpool.tile([128, NE, 256], FP16)
        wv_sb = wpool.tile([128, NE, 256], FP16)
        wo_sb = wpool.tile([128, 2, E], FP16)       # [n%128, pair, e]

        qT_sb = actpool.tile([128, 2, S], FP16)     # [(2 heads x d), pair, q]
        kT_sb = actpool.tile([128, 2, S], FP16)
        v_sb = actpool.tile([128, NK, HC, 72], FP16)  # [s%128, k, h, V|1|pad]
        oT_sb = actpool.tile([128, 2, S], FP16)     # [n%128, pair, s]

        # ---- DMA stream (sync queue), priority order ----
        # wk | xk halves 1 | xq[qc0 cols] | xk halves 2 | wv | xv |
        # xq[rest cols] | wo
        prefix_ctx = ExitStack()
        xkpool = prefix_ctx.enter_context(
            tc.tile_pool(name="xk", bufs=8, side="right"))
        wkpool = prefix_ctx.enter_context(
            tc.tile_pool(name="wk", bufs=1, side="right"))
        psA = prefix_ctx.enter_context(
            tc.tile_pool(name="psA", bufs=2, space="PSUM", side="right"))

        # PE p-state warm-up: ~n_warm dependency-free matmuls keep the PE
        # continuously busy through its 3us ramp window so the projections
        # run at full clock.
        warm_ps = psA.tile([128, 512], F32, tag="mm", name="warm")
        for i in range(n_warm):
            nc.tensor.matmul(warm_ps[:], warm_in[:, 0:128], warm_in[:],
                             start=True, stop=True)

        wk_sb = wkpool.tile([128, NE, 256], FP16)
        nc.sync.dma_start(wk_sb[:], wkT.rearrange("(ec p) n -> p ec n", p=128))
        xk = []
        for e in range(NE):
            t = xkpool.tile([128, S], FP16, tag="xk", name=f"xk{e}")
            nc.sync.dma_start(t[:, 0:1024], xkT[e * 128:(e + 1) * 128, 0:1024])
            xk.append(t)
        nc.sync.dma_start(wq_sb[:], wqT.rearrange("(ec p) n -> p ec n", p=128))

        xq_tiles = {}

        def load_xq(qc):
            t = xqpool.tile([128, NE, 512], FP16, tag="xq", name=f"xq{qc}")
            for e in range(NE):
                nc.sync.dma_start(
                    t[:, e, :],
                    xqT[e * 128:(e + 1) * 128, qc * 512:(qc + 1) * 512])
            xq_tiles[qc] = t

        load_xq(0)
        for e in range(NE):
            nc.sync.dma_start(xk[e][:, 1024:S], xkT[e * 128:(e + 1) * 128, 1024:S])
        nc.sync.dma_start(wv_sb[:], wvT.rearrange("(ec p) n -> p ec n", p=128))
        xv = []
        for e in range(NE):
            t = xvpool.tile([128, S], FP16, tag="xv", name=f"xv{e}")
            nc.sync.dma_start(t[:], xvT[e * 128:(e + 1) * 128, :])
            xv.append(t)
        load_xq(1)
        load_xq(2)
        nc.sync.dma_start(wo_sb[:], woT.rearrange("(pr p) j -> p pr j", p=128))
        load_xq(3)

        # ones column of v_ext (before any pv)
        nc.vector.memset(v_sb[:, :, :, 64:65], 1.0)

        # ---- K projection, m-outer: one kT m-tile (4 k-chunks) at a time
        def kproj_m(m):
            tiles = [psA.tile([128, 512], F32, tag="mm", name=f"kp{m}_{n}")
                     for n in range(2)]
            for e in range(NE):
                for nch in range(2):
                    nc.tensor.matmul(
                        tiles[nch][:],
                        wk_sb[:, e, nch * 128:(nch + 1) * 128],
                        xk[e][:, m * 512:(m + 1) * 512],
                        start=(e == 0), stop=(e == NE - 1),
                    )
            for nch in range(2):
                nc.vector.tensor_copy(
                    kT_sb[:, nch, m * 512:(m + 1) * 512], tiles[nch][:])

        def qproj0():
            tiles = [psA.tile([128, 512], F32, tag="mm", name=f"qp0_{n}")
                     for n in range(2)]
            for e in range(NE):
                for nch in range(2):
                    nc.tensor.matmul(
                        tiles[nch][:],
                        wq_sb[:, e, nch * 128:(nch + 1) * 128],
                        xq_tiles[0][:, e, :],
                        start=(e == 0), stop=(e == NE - 1),
                    )
            for nch in range(2):
                nc.vector.tensor_copy(qT_sb[:, nch, 0:512], tiles[nch][:])

        kproj_m(0)
        qproj0()
        kproj_m(1)

        # ---- steady-state pools (8 PSUM banks total; psA 2 banks closes
        # mid-w0 before psX 2 banks opens; psO 2 banks opens in w2) ----
        ptpool = ctx.enter_context(tc.tile_pool(name="pt", bufs=ptbufs))
        rpool = ctx.enter_context(tc.tile_pool(name="recip", bufs=4))
        onpool = ctx.enter_context(tc.tile_pool(name="onorm", bufs=2))
        stpool = ctx.enter_context(tc.tile_pool(name="stage", bufs=2))
        psS = ctx.enter_context(tc.tile_pool(name="psS", bufs=2, space="PSUM"))
        lazy = {}

        def get_psX():
            if "psX" not in lazy:
                lazy["psX"] = ctx.enter_context(
                    tc.tile_pool(name="psX", bufs=2, space="PSUM"))
            return lazy["psX"]

        def get_psO():
            if "psO" not in lazy:
                lazy["psO"] = ctx.enter_context(
                    tc.tile_pool(name="psO", bufs=2, space="PSUM"))
            return lazy["psO"]

        def close_prefix():
            prefix_ctx.close()

        pt_tiles = {}     # (u, k) -> pT tile
        psO_tiles = {}    # u -> [h2=0 tile, h2=1 tile]
        on_tiles = {}     # u -> normalized O tile [128, 4, 2, 64]

        def sc_exp(u, k):
            qc, pair = u >> 1, u & 1
            qs = slice(qc * 512, (qc + 1) * 512)
            ks = slice(k * 128, (k + 1) * 128)
            ps = psS.tile([128, 1024], F32, tag="ss", name=f"ss{u}_{k}")
            nc.tensor.matmul(ps[:, 0:512], kT_sb[0:64, pair, ks],
                             qT_sb[0:64, pair, qs], start=True, stop=True)
            nc.tensor.matmul(ps[:, 512:1024], kT_sb[64:128, pair, ks],
                             qT_sb[64:128, pair, qs], start=True, stop=True)
            pt = ptpool.tile([128, 1024], FP16, tag="pt", name=f"pt{u}_{k}")
            nc.scalar.activation(pt[:], ps[:],
                                 mybir.ActivationFunctionType.Exp, scale=0.125)
            pt_tiles[(u, k)] = pt

        rt_tiles = {}

        def pv_group(u, g):
            """One accumulation group g = (h2, q): 16 k-chunk matmuls into
            psO[h2][:, q], then normalize THAT region immediately (recip +
            per-partition scale on DVE) so the psO region frees per-group.
            Only one group is ever open per PSUM bank."""
            qc, pair = u >> 1, u & 1
            h2, q = g >> 2, g & 3
            if g == 0:
                pool = get_psO()
                psO_tiles[u] = [
                    pool.tile([128, 4, 128], F32, tag="oo", name=f"oo{u}_{hh}")
                    for hh in range(2)]
                on_tiles[u] = onpool.tile([128, 4, 2, 64], FP16, tag="on",
                                          name=f"on{u}")
                rt_tiles[u] = rpool.tile([128, 2, 4, 1], F32, tag="rt",
                                         name=f"rt{u}")
            h = pair * 2 + h2
            po = psO_tiles[u][h2]
            for k in range(NK):
                nc.tensor.matmul(
                    po[:, q, 0:65],
                    pt_tiles[(u, k)][:, h2 * 512 + q * 128:h2 * 512 + (q + 1) * 128],
                    v_sb[:, k, h, 0:65],
                    start=(k == 0), stop=(k == NK - 1),
                )
            if q == 3:
                # bank h2 complete: one recip + one broadcast multiply
                rt = rt_tiles[u]
                nc.vector.reciprocal(rt[:, h2, :, 0], po[:, :, 64:65])
                in0, in1 = broadcast_tensor_aps(po[:, :, 0:64], rt[:, h2, :, :])
                nc.vector.tensor_tensor(
                    on_tiles[u][:, :, h2, :], in0, in1, mybir.AluOpType.mult)
            if g == 7:
                for k in range(NK):
                    pt_tiles.pop((u, k))
                del psO_tiles[u]
                del rt_tiles[u]

        def transp(u, oT_split=1):
            """PE transpose of O_norm [q, (h2 d)] -> O.T; per-unit psT tile.
            oT_split > 1 splits the psT->oT copy so out-proj can chase."""
            qc, pair = u >> 1, u & 1
            pst = get_psX().tile([128, 4, 128], FP16, tag="px", name=f"tp{u}")
            on = on_tiles.pop(u)
            for q in range(4):
                nc.tensor.transpose(pst[:, q, :], on[:, q, :, :], ident[:])
                if oT_split > 1:
                    nc.vector.tensor_copy(
                        oT_sb[:, pair, qc * 512 + q * 128:qc * 512 + (q + 1) * 128],
                        pst[:, q, :])
            if oT_split == 1:
                nc.vector.tensor_copy(
                    oT_sb[:, pair, qc * 512:(qc + 1) * 512], pst[:, :, :])

        def vproj(m):
            ps = get_psX().tile([128, 512], F32, tag="px", name=f"vp{m}")
            for e in range(NE):
                nc.tensor.matmul(
                    ps[:, 0:256],
                    xv[e][:, m * 128:(m + 1) * 128],
                    wv_sb[:, e, :],
                    start=(e == 0), stop=(e == NE - 1),
                )
            nc.vector.tensor_copy(
                v_sb[:, m, :, 0:64],
                ps[:, 0:256].rearrange("p (h c) -> p h c", h=HC))

        def qproj(qc):
            qs = slice(qc * 512, (qc + 1) * 512)
            tiles = [get_psX().tile([128, 512], F32, tag="px", name=f"qp{qc}_{n}")
                     for n in range(2)]
            for e in range(NE):
                for nch in range(2):
                    nc.tensor.matmul(
                        tiles[nch][:],
                        wq_sb[:, e, nch * 128:(nch + 1) * 128],
                        xq_tiles[qc][:, e, :],
                        start=(e == 0), stop=(e == NE - 1),
                    )
            for nch in range(2):
                nc.vector.tensor_copy(qT_sb[:, nch, qs], tiles[nch][:])

        stage_tiles = {}
        op_ps = {}

        def op_mm(qc, m, j):
            """Out-proj matmuls only; PSUM tile alternates psX/psO pools so
            4 tiles can be in flight before a copy must land."""
            mm = qc * 4 + m
            i = m * 2 + j
            pool = get_psX() if (i & 1) == 0 else get_psO()
            tag = "px" if (i & 1) == 0 else "oo"
            ps = pool.tile([128, 512], F32, tag=tag, name=f"op{mm}_{j}")
            for pair in range(2):
                nc.tensor.matmul(
                    ps[:],
                    oT_sb[:, pair, mm * 128:(mm + 1) * 128],
                    wo_sb[:, pair, j * 512:(j + 1) * 512],
                    start=(pair == 0), stop=(pair == 1),
                )
            op_ps[(mm, j)] = ps

        def op_fin(qc, m, j, copy_eng="vector", dma_eng="sync"):
            """Stage copy (emitted ~2 slots after op_mm so it never blocks
            the DVE queue head) + output DMA."""
            mm = qc * 4 + m
            ps = op_ps.pop((mm, j))
            if j == 0:
                stage_tiles[mm] = stpool.tile([128, E], FP16, tag="st",
                                              name=f"st{mm}")
            dst = stage_tiles[mm][:, j * 512:(j + 1) * 512]
            if copy_eng == "scalar":
                nc.scalar.activation(dst, ps[:],
                                     mybir.ActivationFunctionType.Copy)
            else:
                nc.vector.tensor_copy(dst, ps[:])
            if j == 1:
                getattr(nc, dma_eng).dma_start(
                    out[mm * 128:(mm + 1) * 128, :], stage_tiles.pop(mm)[:])

        # ---- window schedule ----
        # pre-extras run BEFORE the slot's scores (past-gated work: pv
        # groups, transposes, out-proj matmuls) so the in-order PE queue
        # isn't blocked behind the exp-paced scores matmul.  post-extras run
        # after (DMA-gated work: vproj, qproj, Kproj m2/m3).
        pre = {u: {} for u in range(NU)}
        post = {u: {} for u in range(NU)}

        def putq(d, u, slot, fn):
            d[u].setdefault(slot, []).append(fn)

        def put_outproj(w, qc, s0=6):
            """op matmuls at slots s0..s0+7; copies lag 2 slots behind."""
            for i in range(8):
                m, j = i >> 1, i & 1
                putq(pre, w, s0 + i, lambda q=qc, m=m, j=j: op_mm(q, m, j))
                putq(pre, w, s0 + i + 2, lambda q=qc, m=m, j=j: op_fin(q, m, j))

        putq(post, 0, 2, lambda: kproj_m(2))
        putq(post, 0, 6, lambda: kproj_m(3))
        putq(post, 0, 8, close_prefix)
        for i, m in enumerate(range(0, 4)):      # vproj: xv lands ~slot 11
            putq(post, 0, 12 + i, lambda m=m: vproj(m))
        for i, m in enumerate(range(4, 16)):
            putq(post, 1, 0 + i, lambda m=m: vproj(m))
        putq(post, 1, 13, lambda: qproj(1))
        putq(pre, 3, 0, lambda: transp(0))
        putq(post, 3, 13, lambda: qproj(2))
        putq(pre, 4, 0, lambda: transp(1))
        putq(pre, 4, 2, lambda: transp(2))
        put_outproj(4, 0)
        putq(post, 4, 14, lambda: qproj(3))
        putq(pre, 5, 0, lambda: transp(3))
        put_outproj(5, 1)
        putq(pre, 6, 0, lambda: transp(4))
        putq(pre, 7, 0, lambda: transp(5))
        put_outproj(7, 2)

        # pv groups per window: dense in slots 0-7 (one group per slot) so
        # psO regions + pt tiles free early; w3 catches up with u1 then u2.
        def pv_items(u, k):
            if u == 2:
                # 2 groups/slot: frees u0's pt tiles by ~slot 4, before
                # exp(u2, k>=3) needs their pool slots (ptbufs=35)
                return [(0, 2 * k), (0, 2 * k + 1)] if k < 4 else []
            if u == 3:
                return [(1, k)] if k < 8 else [(2, k - 8)]
            if u >= 4:
                return [(u - 1, k)] if k < 8 else []
            return []

        for u in range(NU):
            for k in range(NK):
                for (pu, g) in pv_items(u, k):
                    pv_group(pu, g)
                for fn in pre[u].get(k, ()):
                    fn()
                sc_exp(u, k)
                for fn in post[u].get(k, ()):
                    fn()

        # ---- tail: pv(u7) after the last exps; transposes then qc3 out-proj
        # with copies alternating DVE/ACT and DMA queues.
        if debug:
            nc.sync.dma_start(dbg_pt[:, :], pt_tiles[(7, NK - 1)][:])
        for g in range(4):
            pv_group(7, g)
        transp(6)
        for g in range(4, 8):
            pv_group(7, g)
        transp(7, oT_split=4)
        items = [(m, j) for m in range(4) for j in range(2)]
        for i, (m, j) in enumerate(items):
            op_mm(3, m, j)
            if i >= 1:
                pm, pj = items[i - 1]
                op_fin(3, pm, pj,
                       copy_eng=("vector" if pj == 0 else "scalar"),
                       dma_eng=("sync" if (pm & 1) == 0 else "gpsimd"))
        op_fin(3, 3, 1, copy_eng="scalar", dma_eng="gpsimd")

        if debug:
            nc.sync.dma_start(dbg_qT[:, :, :], qT_sb[:])
            nc.sync.dma_start(dbg_kT[:, :, :], kT_sb[:])
            nc.sync.dma_start(dbg_v[:, :, :, :], v_sb[:])
            nc.sync.dma_start(dbg_oT[:, :, :], oT_sb[:])

    return nc


_NC_CACHE = {}


def _get_nc():
    if "nc" not in _NC_CACHE:
        _NC_CACHE["nc"] = build()
    return _NC_CACHE["nc"]


def _shard_inputs(query, key, value, Wq, Wk, Wv, Wo):
    """Host-side sharding + layout prep: core c = (batch c//4, head-group c%4)."""
    f16 = np.float16
    xT = []
    for b in range(B):
        xT.append((
            np.ascontiguousarray(query[b].T).astype(f16),
            np.ascontiguousarray(key[b].T).astype(f16),
            np.ascontiguousarray(value[b].T).astype(f16),
        ))
    wT = []
    for g in range(4):
        gc = slice(g * 256, (g + 1) * 256)
        wT.append((
            np.ascontiguousarray(Wq[gc].T).astype(f16),
            np.ascontiguousarray(Wk[gc].T).astype(f16),
            np.ascontiguousarray(Wv[gc].T).astype(f16),
            np.ascontiguousarray(Wo[:, gc].T).astype(f16),
        ))
    in_maps = []
    for c in range(NCORES):
        b, g = c // 4, c % 4
        qT, kT, vT = xT[b]
        wq, wk, wv, wo = wT[g]
        in_maps.append({
            "xqT": qT, "xkT": kT, "xvT": vT,
            "wqT": wq, "wkT": wk, "wvT": wv, "woT": wo,
        })
    return in_maps


def kernel(query, key, value, Wq, Wk, Wv, Wo):
    query = np.asarray(query, dtype=np.float32)
    key = np.asarray(key, dtype=np.float32)
    value = np.asarray(value, dtype=np.float32)
    Wq = np.asarray(Wq, dtype=np.float32)
    Wk = np.asarray(Wk, dtype=np.float32)
    Wv = np.asarray(Wv, dtype=np.float32)
    Wo = np.asarray(Wo, dtype=np.float32)

    nc = _get_nc()
    in_maps = _shard_inputs(query, key, value, Wq, Wk, Wv, Wo)
    res = run_bass_kernel_spmd(nc, in_maps, core_ids=list(range(NCORES)))

    out = np.zeros((B, S, E), dtype=np.float32)
    for c in range(NCORES):
        out[c // 4] += res.results[c]["out"].astype(np.float32)
    return out


# revision 52
# speedup vs baseline: 1.4228x; 1.0366x over previous
"""Trainium2 Bass kernel for nn_MultiHeadAttention (B=2, S=2048, E=1024, H=16).

Sharding: 8 NeuronCores = data-parallel over the 2 batches x tensor-parallel
over the 16 heads in 4 groups of 4 heads (Wq/Wk/Wv split column-wise, Wo
row-wise).  Each core computes a full-[S, E] partial of its batch's output;
the host sums the 4 head-group partials per batch.

Per-core device algorithm (v2 — stationary-P orientation):
  K.T/Q.T[n, s]  = (wT chunk).T @ xT chunk       e-outer, chases input DMAs
  V[s, n]        = (xvT chunk).T @ wvT chunk     stored [s%128, k, h, V|1|pad]
  S.T_h[k, q]    = (K_h.T chunk).T @ Q_h.T       row-packed head pairs (d=64)
  P.T            = exp(S.T / 8)                  fp16, one ACT op per (k, pair)
  [O_h | rowsum] = (P.T chunk as STATIONARY).T @ [V_h | 1]
                   — moving dim is only 65 wide, so P@V costs half the
                   PE rows of the d-moving orientation
  O_hn           = O_h * recip(rowsum)           per-partition scalar on DVE
  O.T            = PE-transpose(O_hn)            fp16, 128 rows per 64x128 tile
  out[m, :]      = sum_pair (O.T pair chunk).T @ Wo.T pair chunk
                   — 2-head-packed contraction (K=128)

Everything fp16 except PSUM accumulation (fp32) and the reciprocal.  The
output partial is returned fp16 and summed on the host in fp32.
"""

import numpy as np
from contextlib import ExitStack

import concourse.bass as bass
import concourse.mybir as mybir
import concourse.tile as tile
from concourse.tile import ScopedClock
from concourse.bass import broadcast_tensor_aps
from concourse.bass_utils import run_bass_kernel_spmd
from concourse.masks import make_identity

# ---------------------------------------------------------------------------
# Workarounds for the walrus build on this stack, which rejects more than ONE
# semaphore wait per instruction ("Too many sync wait commands").
# ---------------------------------------------------------------------------
_orig_commit_instruction = tile.TileContext._commit_instruction


def _commit_instruction(self, inst, lazy_reg_writes=True):
    si = getattr(inst, "sync_info", None)
    if si is not None and si.on_wait and len(si.on_wait) > 1:
        waits = list(si.on_wait)
        for w in waits[:-1]:
            nop = mybir.InstNoOp(
                name=self.nc.get_next_instruction_name(),
                ins=[], outs=[], engine=inst.engine,
            )
            nop.bass_nofuse = True
            nop.sync_info = mybir.SyncInfo(on_wait=[w], on_update=[])
            _orig_commit_instruction(self, nop, lazy_reg_writes=False)
        inst.sync_info = mybir.SyncInfo(
            on_wait=[waits[-1]], on_update=list(si.on_update or [])
        )
    return _orig_commit_instruction(self, inst, lazy_reg_writes)


def _drain_and_barrier(self, tick_clock, wait_clock):
    nc = self.nc
    drain_inst = nc.sync.drain()
    wait_clock.add_sem_waits(
        drain_inst.ins, ScopedClock({None: tick_clock.global_clock})
    )
    si = drain_inst.ins.sync_info
    waits = list(si.on_wait) if si and si.on_wait else []
    if len(waits) > 1:
        drain_inst.ins.sync_info = mybir.SyncInfo(
            on_wait=waits[:1], on_update=list(si.on_update or [])
        )
        for w in waits[1:]:
            extra = nc.sync.drain()
            esi = extra.ins.sync_info
            extra.ins.sync_info = mybir.SyncInfo(
                on_wait=[w],
                on_update=list(esi.on_update or []) if esi else [],
            )
    nc.all_engine_barrier()
    assert self.sems is not None
    popped = nc._tile_sem_poison_stack.pop()
    assert popped is self._sem_poison
    nc.clear_and_free_semaphores(list(self.sems.allocated().values()))
    nc.all_engine_barrier()


def _apply_tilefix():
    tile.TileContext._commit_instruction = _commit_instruction
    tile.TileContext._drain_and_barrier = _drain_and_barrier


_apply_tilefix()

# ---------------------------------------------------------------------------
# Problem constants (hardcoded)
# ---------------------------------------------------------------------------
B, S, E, H = 2, 2048, 1024, 16
HC, D = 4, 64              # heads per core, head dim
NCORES = 8
NE = E // 128              # 8  e-chunks
NQC = S // 512             # 4  qc windows
NK = S // 128              # 16 k-chunks (also m-chunks)
NU = 2 * NQC               # 8 units = (qc, pair)

F32 = mybir.dt.float32
FP16 = mybir.dt.float16


def build(ptbufs=35, n_warm=(12, 30), debug=False):
    nc = bass.Bass()
    xqT = nc.dram_tensor("xqT", [E, S], FP16, kind="ExternalInput")
    xkT = nc.dram_tensor("xkT", [E, S], FP16, kind="ExternalInput")
    xvT = nc.dram_tensor("xvT", [E, S], FP16, kind="ExternalInput")
    wqT = nc.dram_tensor("wqT", [E, 256], FP16, kind="ExternalInput")
    wkT = nc.dram_tensor("wkT", [E, 256], FP16, kind="ExternalInput")
    wvT = nc.dram_tensor("wvT", [E, 256], FP16, kind="ExternalInput")
    woT = nc.dram_tensor("woT", [256, E], FP16, kind="ExternalInput")
    out = nc.dram_tensor("out", [S, E], FP16, kind="ExternalOutput")
    if debug:
        dbg_qT = nc.dram_tensor("dbg_qT", [128, 2, S], FP16, kind="ExternalOutput")
        dbg_kT = nc.dram_tensor("dbg_kT", [128, 2, S], FP16, kind="ExternalOutput")
        dbg_v = nc.dram_tensor("dbg_v", [128, NK, HC, 72], FP16, kind="ExternalOutput")
        dbg_oT = nc.dram_tensor("dbg_oT", [128, 2, S], FP16, kind="ExternalOutput")
        dbg_pt = nc.dram_tensor("dbg_pt", [128, 1024], FP16, kind="ExternalOutput")
        dbg_on = nc.dram_tensor("dbg_on", [128, 4, 2, 64], FP16, kind="ExternalOutput")
        dbg_po = nc.dram_tensor("dbg_po", [128, 2, 4, 16], F32, kind="ExternalOutput")

    with tile.TileContext(nc) as tc, ExitStack() as ctx:
        consts = ctx.enter_context(tc.tile_pool(name="consts", bufs=1))
        wpool = ctx.enter_context(tc.tile_pool(name="w", bufs=1))
        actpool = ctx.enter_context(tc.tile_pool(name="acts", bufs=1))
        xqpool = ctx.enter_context(tc.tile_pool(name="xq", bufs=2))
        xvpool = ctx.enter_context(tc.tile_pool(name="xv", bufs=8))

        # exp table preload + transpose identity
        dummy = consts.tile([1, 8], F32)
        nc.vector.memset(dummy[:], 0.0)
        nc.scalar.activation(dummy[:], dummy[:], mybir.ActivationFunctionType.Exp)
        ident = consts.tile([128, 128], FP16)
        make_identity(nc, ident)
        warm_in = consts.tile([128, 512], FP16)
        nc.gpsimd.memset(warm_in[:], 0.0)

        wq_sb = wpool.tile([128, NE, 256], FP16)
        wv_sb = wpool.tile([128, NE, 256], FP16)
        wo_sb = wpool.tile([128, 2, E], FP16)       # [n%128, pair, e]

        qT_sb = actpool.tile([128, 2, S], FP16)     # [(2 heads x d), pair, q]
        kT_sb = actpool.tile([128, 2, S], FP16)
        v_sb = actpool.tile([128, NK, HC, 72], FP16)  # [s%128, k, h, V|1|pad]
        oT_sb = actpool.tile([128, 2, S], FP16)     # [n%128, pair, s]

        # ---- DMA stream (sync queue), priority order ----
        # wk | xk halves 1 | xq[qc0 cols] | xk halves 2 | wv | xv |
        # xq[rest cols] | wo
        prefix_ctx = ExitStack()
        xkpool = prefix_ctx.enter_context(
            tc.tile_pool(name="xk", bufs=8, side="right"))
        wkpool = prefix_ctx.enter_context(
            tc.tile_pool(name="wk", bufs=1, side="right"))
        psA = prefix_ctx.enter_context(
            tc.tile_pool(name="psA", bufs=2, space="PSUM", side="right"))

        # PE p-state warm-up: ~n_warm dependency-free matmuls keep the PE
        # continuously busy through its 3us ramp window so the projections
        # run at full clock.
        warm_ps = psA.tile([128, 512], F32, tag="mm", name="warm")
        n_big, n_tiny = (n_warm if isinstance(n_warm, tuple) else (n_warm, 0))
        for i in range(n_big):
            nc.tensor.matmul(warm_ps[:], warm_in[:, 0:128], warm_in[:],
                             start=True, stop=True)
        for i in range(n_tiny):
            nc.tensor.matmul(warm_ps[:, 0:32], warm_in[:, 0:128],
                             warm_in[:, 0:32], start=True, stop=True)

        wk_sb = wkpool.tile([128, NE, 256], FP16)
        nc.sync.dma_start(wk_sb[:], wkT.rearrange("(ec p) n -> p ec n", p=128))
        xk = []
        for e in range(NE):
            t = xkpool.tile([128, S], FP16, tag="xk", name=f"xk{e}")
            nc.sync.dma_start(t[:, 0:1024], xkT[e * 128:(e + 1) * 128, 0:1024])
            xk.append(t)
        nc.sync.dma_start(wq_sb[:], wqT.rearrange("(ec p) n -> p ec n", p=128))

        xq_tiles = {}

        def load_xq(qc):
            t = xqpool.tile([128, NE, 512], FP16, tag="xq", name=f"xq{qc}")
            for e in range(NE):
                nc.sync.dma_start(
                    t[:, e, :],
                    xqT[e * 128:(e + 1) * 128, qc * 512:(qc + 1) * 512])
            xq_tiles[qc] = t

        load_xq(0)
        for e in range(NE):
            nc.sync.dma_start(xk[e][:, 1024:S], xkT[e * 128:(e + 1) * 128, 1024:S])
        nc.sync.dma_start(wv_sb[:], wvT.rearrange("(ec p) n -> p ec n", p=128))
        xv = []
        for e in range(NE):
            t = xvpool.tile([128, S], FP16, tag="xv", name=f"xv{e}")
            nc.sync.dma_start(t[:], xvT[e * 128:(e + 1) * 128, :])
            xv.append(t)
        load_xq(1)
        load_xq(2)
        nc.sync.dma_start(wo_sb[:], woT.rearrange("(pr p) j -> p pr j", p=128))
        load_xq(3)

        # ones column of v_ext (before any pv)
        nc.vector.memset(v_sb[:, :, :, 64:65], 1.0)

        # ---- K projection, m-outer: one kT m-tile (4 k-chunks) at a time;
        # emission can be chunked (i0..i1 of the 16 (e, nch) matmuls) to
        # interleave with the first score window.
        kp_tiles = {}

        def kproj_mms(m, i0, i1):
            if m not in kp_tiles:
                kp_tiles[m] = [
                    psA.tile([128, 512], F32, tag="mm", name=f"kp{m}_{n}")
                    for n in range(2)]
            tiles = kp_tiles[m]
            for i in range(i0, i1):
                e, nch = i >> 1, i & 1
                nc.tensor.matmul(
                    tiles[nch][:],
                    wk_sb[:, e, nch * 128:(nch + 1) * 128],
                    xk[e][:, m * 512:(m + 1) * 512],
                    start=(e == 0), stop=(e == NE - 1),
                )
            if i1 == 2 * NE:
                for nch in range(2):
                    nc.vector.tensor_copy(
                        kT_sb[:, nch, m * 512:(m + 1) * 512], tiles[nch][:])
                del kp_tiles[m]

        def kproj_m(m):
            kproj_mms(m, 0, 2 * NE)

        def qproj0():
            tiles = [psA.tile([128, 512], F32, tag="mm", name=f"qp0_{n}")
                     for n in range(2)]
            for e in range(NE):
                for nch in range(2):
                    nc.tensor.matmul(
                        tiles[nch][:],
                        wq_sb[:, e, nch * 128:(nch + 1) * 128],
                        xq_tiles[0][:, e, :],
                        start=(e == 0), stop=(e == NE - 1),
                    )
            for nch in range(2):
                nc.vector.tensor_copy(qT_sb[:, nch, 0:512], tiles[nch][:])

        kproj_m(0)
        qproj0()
        kproj_m(1)

        # ---- steady-state pools (8 PSUM banks total; psA 2 banks closes
        # mid-w0 before psX 2 banks opens; psO 2 banks opens in w2) ----
        ptpool = ctx.enter_context(tc.tile_pool(name="pt", bufs=ptbufs))
        rpool = ctx.enter_context(tc.tile_pool(name="recip", bufs=4))
        onpool = ctx.enter_context(tc.tile_pool(name="onorm", bufs=2))
        stpool = ctx.enter_context(tc.tile_pool(name="stage", bufs=2))
        psS = ctx.enter_context(tc.tile_pool(name="psS", bufs=2, space="PSUM"))
        lazy = {}

        def get_psX():
            if "psX" not in lazy:
                lazy["psX"] = ctx.enter_context(
                    tc.tile_pool(name="psX", bufs=2, space="PSUM"))
            return lazy["psX"]

        def get_psO():
            if "psO" not in lazy:
                lazy["psO"] = ctx.enter_context(
                    tc.tile_pool(name="psO", bufs=2, space="PSUM"))
            return lazy["psO"]

        def close_prefix():
            prefix_ctx.close()

        pt_tiles = {}     # (u, k) -> pT tile
        psO_tiles = {}    # u -> [h2=0 tile, h2=1 tile]
        on_tiles = {}     # u -> normalized O tile [128, 4, 2, 64]
        transp_ps = {}    # u -> psT tile

        def sc_exp(u, k):
            qc, pair = u >> 1, u & 1
            qs = slice(qc * 512, (qc + 1) * 512)
            ks = slice(k * 128, (k + 1) * 128)
            ps = psS.tile([128, 1024], F32, tag="ss", name=f"ss{u}_{k}")
            nc.tensor.matmul(ps[:, 0:512], kT_sb[0:64, pair, ks],
                             qT_sb[0:64, pair, qs], start=True, stop=True)
            nc.tensor.matmul(ps[:, 512:1024], kT_sb[64:128, pair, ks],
                             qT_sb[64:128, pair, qs], start=True, stop=True)
            pt = ptpool.tile([128, 1024], FP16, tag="pt", name=f"pt{u}_{k}")
            nc.scalar.activation(pt[:], ps[:],
                                 mybir.ActivationFunctionType.Exp, scale=0.125)
            pt_tiles[(u, k)] = pt

        rt_tiles = {}
        pv_done = {}

        def pv_group(u, g, eager=False):
            """One accumulation group g = (h2, q): 16 k-chunk matmuls into
            psO[h2][:, q], then normalize THAT region immediately (recip +
            per-partition scale on DVE) so the psO region frees per-group.
            Only one group is ever open per PSUM bank."""
            qc, pair = u >> 1, u & 1
            h2, q = g >> 2, g & 3
            if g == 0:
                pool = get_psO()
                psO_tiles[u] = [
                    pool.tile([128, 4, 128], F32, tag="oo", name=f"oo{u}_{hh}")
                    for hh in range(2)]
                on_tiles[u] = onpool.tile([128, 4, 2, 64], FP16, tag="on",
                                          name=f"on{u}")
                rt_tiles[u] = rpool.tile([128, 2, 4, 1], F32, tag="rt",
                                         name=f"rt{u}")
            h = pair * 2 + h2
            po = psO_tiles[u][h2]
            for k in range(NK):
                nc.tensor.matmul(
                    po[:, q, 0:65],
                    pt_tiles[(u, k)][:, h2 * 512 + q * 128:h2 * 512 + (q + 1) * 128],
                    v_sb[:, k, h, 0:65],
                    start=(k == 0), stop=(k == NK - 1),
                )
            if debug and u == 0 and g == 7:
                dbg_sb = consts.tile([128, 2, 4, 16], F32, name=f"dbgpo{g}")
                for hh in range(2):
                    nc.vector.tensor_copy(dbg_sb[:, hh],
                                          psO_tiles[u][hh][:, :, 56:72])
                nc.sync.dma_start(dbg_po[:], dbg_sb[:])
            if eager:
                # tail: normalize this (h2, q) region immediately so the
                # transpose/out-proj chain can chase per q-slot
                rt = rt_tiles[u]
                nc.vector.reciprocal(rt[:, h2, q:q + 1, 0], po[:, q, 64:65])
                nc.vector.tensor_scalar_mul(
                    on_tiles[u][:, q, h2, :], po[:, q, 0:64],
                    rt[:, h2, q:q + 1, 0])
            elif q == 3:
                # bank h2 complete: one recip + one broadcast multiply
                rt = rt_tiles[u]
                nc.vector.reciprocal(rt[:, h2, :, 0], po[:, :, 64:65])
                in0, in1 = broadcast_tensor_aps(po[:, :, 0:64], rt[:, h2, :, :])
                nc.vector.tensor_tensor(
                    on_tiles[u][:, :, h2, :], in0, in1, mybir.AluOpType.mult)
            done_groups = pv_done.setdefault(u, set())
            done_groups.add(g)
            if len(done_groups) == 8:
                for k in range(NK):
                    pt_tiles.pop((u, k))
                del psO_tiles[u]
                del rt_tiles[u]

        def transp(u, oT_split=1, qs_list=None):
            """PE transpose of O_norm [q, (h2 d)] -> O.T; per-unit psT tile.
            oT_split > 1 splits the psT->oT copy so out-proj can chase."""
            qc, pair = u >> 1, u & 1
            if u not in transp_ps:
                transp_ps[u] = get_psX().tile([128, 4, 128], FP16, tag="px",
                                              name=f"tp{u}")
            pst = transp_ps[u]
            on = on_tiles[u]
            for q in (qs_list if qs_list is not None else range(4)):
                nc.tensor.transpose(pst[:, q, :], on[:, q, :, :], ident[:])
                if oT_split > 1:
                    nc.vector.tensor_copy(
                        oT_sb[:, pair, qc * 512 + q * 128:qc * 512 + (q + 1) * 128],
                        pst[:, q, :])
            if oT_split == 1:
                nc.vector.tensor_copy(
                    oT_sb[:, pair, qc * 512:(qc + 1) * 512], pst[:, :, :])

        def vproj(m):
            ps = get_psX().tile([128, 512], F32, tag="px", name=f"vp{m}")
            for e in range(NE):
                nc.tensor.matmul(
                    ps[:, 0:256],
                    xv[e][:, m * 128:(m + 1) * 128],
                    wv_sb[:, e, :],
                    start=(e == 0), stop=(e == NE - 1),
                )
            nc.vector.tensor_copy(
                v_sb[:, m, :, 0:64],
                ps[:, 0:256].rearrange("p (h c) -> p h c", h=HC))

        def qproj(qc):
            qs = slice(qc * 512, (qc + 1) * 512)
            tiles = [get_psX().tile([128, 512], F32, tag="px", name=f"qp{qc}_{n}")
                     for n in range(2)]
            for e in range(NE):
                for nch in range(2):
                    nc.tensor.matmul(
                        tiles[nch][:],
                        wq_sb[:, e, nch * 128:(nch + 1) * 128],
                        xq_tiles[qc][:, e, :],
                        start=(e == 0), stop=(e == NE - 1),
                    )
            for nch in range(2):
                nc.vector.tensor_copy(qT_sb[:, nch, qs], tiles[nch][:])

        stage_tiles = {}
        op_ps = {}

        def op_mm(qc, m, j):
            """Out-proj matmuls only; PSUM tile alternates psX/psO pools so
            4 tiles can be in flight before a copy must land."""
            mm = qc * 4 + m
            i = m * 2 + j
            pool = get_psX() if (i & 1) == 0 else get_psO()
            tag = "px" if (i & 1) == 0 else "oo"
            ps = pool.tile([128, 512], F32, tag=tag, name=f"op{mm}_{j}")
            for pair in range(2):
                nc.tensor.matmul(
                    ps[:],
                    oT_sb[:, pair, mm * 128:(mm + 1) * 128],
                    wo_sb[:, pair, j * 512:(j + 1) * 512],
                    start=(pair == 0), stop=(pair == 1),
                )
            op_ps[(mm, j)] = ps

        def op_fin(qc, m, j, copy_eng="vector", dma_eng="sync"):
            """Stage copy (emitted ~2 slots after op_mm so it never blocks
            the DVE queue head) + output DMA."""
            mm = qc * 4 + m
            ps = op_ps.pop((mm, j))
            if j == 0:
                stage_tiles[mm] = stpool.tile([128, E], FP16, tag="st",
                                              name=f"st{mm}")
            dst = stage_tiles[mm][:, j * 512:(j + 1) * 512]
            if copy_eng == "scalar":
                nc.scalar.activation(dst, ps[:],
                                     mybir.ActivationFunctionType.Copy)
            else:
                nc.vector.tensor_copy(dst, ps[:])
            if j == 1:
                getattr(nc, dma_eng).dma_start(
                    out[mm * 128:(mm + 1) * 128, :], stage_tiles.pop(mm)[:])

        # ---- window schedule ----
        # pre-extras run BEFORE the slot's scores (past-gated work: pv
        # groups, transposes, out-proj matmuls) so the in-order PE queue
        # isn't blocked behind the exp-paced scores matmul.  post-extras run
        # after (DMA-gated work: vproj, qproj, Kproj m2/m3).
        pre = {u: {} for u in range(NU)}
        post = {u: {} for u in range(NU)}

        def putq(d, u, slot, fn):
            d[u].setdefault(slot, []).append(fn)

        def put_outproj(w, qc):
            """Even items (psX) at slots 2,4,6,8 with fins at +3; odd items
            (psO) at 8,10,12,14 with fins at +2.  Every PSUM-slot WAR then
            has >=1.7 slots of slack, so out-proj never back-pressures the
            pv/exp pipeline."""
            for i in range(8):
                m, j = i >> 1, i & 1
                if (i & 1) == 0:
                    s, f = 2 + i, 5 + i
                else:
                    s, f = 7 + i, min(15, 9 + i)
                putq(pre, w, s, lambda q=qc, m=m, j=j: op_mm(q, m, j))
                putq(pre, w, f, lambda q=qc, m=m, j=j: op_fin(q, m, j))

        putq(post, 0, 2, lambda: kproj_m(2))
        putq(post, 0, 6, lambda: kproj_m(3))
        putq(post, 0, 8, close_prefix)
        for i, m in enumerate(range(0, 4)):      # vproj: xv lands ~slot 11
            putq(post, 0, 12 + i, lambda m=m: vproj(m))
        for i, m in enumerate(range(4, 16)):
            putq(post, 1, 0 + i, lambda m=m: vproj(m))
        putq(post, 1, 13, lambda: qproj(1))
        putq(pre, 3, 0, lambda: transp(0))
        if debug:
            putq(pre, 3, 1, lambda: nc.sync.dma_start(dbg_on[:], on_tiles[0][:]))
        putq(post, 3, 13, lambda: qproj(2))
        putq(pre, 4, 0, lambda: transp(1))
        putq(pre, 4, 2, lambda: transp(2))
        put_outproj(4, 0)
        putq(post, 4, 14, lambda: qproj(3))
        putq(pre, 5, 0, lambda: transp(3))
        put_outproj(5, 1)
        putq(pre, 6, 0, lambda: transp(4))
        putq(pre, 7, 0, lambda: transp(5))
        put_outproj(7, 2)

        # pv groups per window: dense in slots 0-7 (one group per slot) so
        # psO regions + pt tiles free early; w3 catches up with u1 then u2.
        def pv_items(u, k):
            if u == 2:
                # 2 groups/slot: frees u0's pt tiles by ~slot 4, before
                # exp(u2, k>=3) needs their pool slots (ptbufs=35)
                return [(0, 2 * k), (0, 2 * k + 1)] if k < 4 else []
            if u == 3:
                return [(1, k)] if k < 8 else [(2, k - 8)]
            if u >= 4:
                return [(u - 1, k)] if k < 8 else []
            return []

        for u in range(NU):
            for k in range(NK):
                for (pu, g) in pv_items(u, k):
                    pv_group(pu, g)
                for fn in pre[u].get(k, ()):
                    fn()
                sc_exp(u, k)
                for fn in post[u].get(k, ()):
                    fn()

        # ---- tail: per-q-slot chase. For each q: both banks' pv groups
        # (eagerly normalized), the q transpose + oT copy, then that m's
        # out-proj matmuls; fins lag one iteration. All PSUM tiles come from
        # psX in strict rotation so every WAR is against already-emitted ops.
        if debug:
            nc.sync.dma_start(dbg_pt[:, :], pt_tiles[(7, NK - 1)][:])
        transp(6)

        def tail_transp_q(q):
            pst = get_psX().tile([128, 128], FP16, tag="px", name=f"tp7_{q}")
            nc.tensor.transpose(pst[:, :], on_tiles[7][:, q, :, :], ident[:])
            nc.vector.tensor_copy(
                oT_sb[:, 1, 3 * 512 + q * 128:3 * 512 + (q + 1) * 128], pst[:])

        def tail_op_mm(m, j):
            ps = get_psX().tile([128, 512], F32, tag="px", name=f"top{m}_{j}")
            mm = 12 + m
            for pair in range(2):
                nc.tensor.matmul(
                    ps[:],
                    oT_sb[:, pair, mm * 128:(mm + 1) * 128],
                    wo_sb[:, pair, j * 512:(j + 1) * 512],
                    start=(pair == 0), stop=(pair == 1),
                )
            op_ps[(mm, j)] = ps

        for q in range(4):
            pv_group(7, q, eager=True)
            pv_group(7, 4 + q, eager=True)
            if q > 0:
                for j in range(2):
                    op_fin(3, q - 1, j,
                           copy_eng=("vector" if j == 0 else "scalar"))
            tail_transp_q(q)
            for j in range(2):
                tail_op_mm(q, j)
        for j in range(2):
            op_fin(3, 3, j, copy_eng=("vector" if j == 0 else "scalar"))

        if debug:
            nc.sync.dma_start(dbg_qT[:, :, :], qT_sb[:])
            nc.sync.dma_start(dbg_kT[:, :, :], kT_sb[:])
            nc.sync.dma_start(dbg_v[:, :, :, :], v_sb[:])
            nc.sync.dma_start(dbg_oT[:, :, :], oT_sb[:])

    return nc


_NC_CACHE = {}


def _get_nc():
    if "nc" not in _NC_CACHE:
        _NC_CACHE["nc"] = build()
    return _NC_CACHE["nc"]


def _shard_inputs(query, key, value, Wq, Wk, Wv, Wo):
    """Host-side sharding + layout prep: core c = (batch c//4, head-group c%4)."""
    f16 = np.float16
    xT = []
    for b in range(B):
        xT.append((
            np.ascontiguousarray(query[b].T).astype(f16),
            np.ascontiguousarray(key[b].T).astype(f16),
            np.ascontiguousarray(value[b].T).astype(f16),
        ))
    wT = []
    for g in range(4):
        gc = slice(g * 256, (g + 1) * 256)
        wT.append((
            np.ascontiguousarray(Wq[gc].T).astype(f16),
            np.ascontiguousarray(Wk[gc].T).astype(f16),
            np.ascontiguousarray(Wv[gc].T).astype(f16),
            np.ascontiguousarray(Wo[:, gc].T).astype(f16),
        ))
    in_maps = []
    for c in range(NCORES):
        b, g = c // 4, c % 4
        qT, kT, vT = xT[b]
        wq, wk, wv, wo = wT[g]
        in_maps.append({
            "xqT": qT, "xkT": kT, "xvT": vT,
            "wqT": wq, "wkT": wk, "wvT": wv, "woT": wo,
        })
    return in_maps


def kernel(query, key, value, Wq, Wk, Wv, Wo):
    query = np.asarray(query, dtype=np.float32)
    key = np.asarray(key, dtype=np.float32)
    value = np.asarray(value, dtype=np.float32)
    Wq = np.asarray(Wq, dtype=np.float32)
    Wk = np.asarray(Wk, dtype=np.float32)
    Wv = np.asarray(Wv, dtype=np.float32)
    Wo = np.asarray(Wo, dtype=np.float32)

    nc = _get_nc()
    in_maps = _shard_inputs(query, key, value, Wq, Wk, Wv, Wo)
    res = run_bass_kernel_spmd(nc, in_maps, core_ids=list(range(NCORES)))

    out = np.zeros((B, S, E), dtype=np.float32)
    for c in range(NCORES):
        out[c // 4] += res.results[c]["out"].astype(np.float32)
    return out
